# revision 5
# baseline (speedup 1.0000x reference)
"""MixHop layer (hop0 + A@h1 + A^2@h2) on 8 trn2 NeuronCores.

Strategy: 1D node partition (rows) across 8 cores. Dense hop matmuls on
TensorE. SpMM = dma_gather of neighbor features (bf16, 512B rows) +
one-hot scatter matmuls on TensorE accumulating into per-128-row-window
PSUM tiles; the one-hot-scaled weight tile P_T[e, r] = val_e * (r == off_e)
is built with a single DVE tensor_scalar (is_equal, mult) per 128-edge
chunk. Cross-core halo handled by two AllGathers (hcat=[h1|h2] bf16, g bf16).
"""
import os
import sys

for p in ("/opt/trn_rl_repo", "/root/.axon_site/_ro/trn_rl_repo"):
    if os.path.isdir(p) and p not in sys.path:
        sys.path.append(p)

import numpy as np

N = 50000
E = 600000
C = 128
CORES = 8
RPC = 6272                # rows per core (padded)
NP = RPC * CORES          # 50176
NW = RPC // 128           # 49 windows per core
SG = 7                    # windows per gather supergroup
GROUPS = [(g * SG, min(NW, (g + 1) * SG)) for g in range((NW + SG - 1) // SG)]

TRACE = False             # test.py can flip this for profiling
STAGES = int(os.environ.get("KM_STAGES", "5"))  # 1=dense 2=+AG1 3=+pass1 4=+AG2 5=+pass2
_CACHE = {}


def _build_plan(edge_row, edge_col, edge_val):
    """Host-side edge preprocessing. Returns per-core tables + structure."""
    core = edge_row // RPC
    w = (edge_row % RPC) // 128
    off = (edge_row % 128).astype(np.float32)
    par = (edge_col % 2).astype(np.int64)
    gidx = (edge_col // 2).astype(np.int16)

    gid = (core.astype(np.int64) * NW + w) * 2 + par
    ngroups = CORES * NW * 2
    counts = np.bincount(gid, minlength=ngroups).reshape(CORES, NW, 2)
    # chunks per (window, parity), shared across cores
    Bw = np.maximum(1, ((counts.max(axis=0) + 127) // 128))  # [NW, 2]

    # global chunk layout in "call order": for group: for par: for w in group
    cstart = np.zeros((NW, 2), np.int64)
    calls = []  # dicts: par, ws, cstart, nch
    cpos = 0
    for (w0, w1) in GROUPS:
        for p in (0, 1):
            ws = list(range(w0, w1))
            nch = int(Bw[w0:w1, p].sum())
            for wi in ws:
                cstart[wi, p] = cpos
                cpos += int(Bw[wi, p])
            calls.append(dict(par=p, ws=ws, cstart=cpos - nch, nch=nch))
    T = cpos

    # place each edge: core-local position = cstart[w,par]*128 + rank
    order = np.argsort(gid, kind="stable")
    gs = np.zeros(ngroups + 1, np.int64)
    np.cumsum(counts.reshape(-1), out=gs[1:])
    rank = np.arange(E, dtype=np.int64) - gs[gid[order]]
    pos = cstart[w[order], par[order]] * 128 + rank
    flat = core[order].astype(np.int64) * (T * 128) + pos

    idx_p = np.zeros(CORES * T * 128, np.int16)
    off_p = np.zeros(CORES * T * 128, np.float32)
    val_p = np.zeros(CORES * T * 128, np.float32)
    idx_p[flat] = gidx[order]
    off_p[flat] = off[order]
    val_p[flat] = edge_val[order]
    idx_p = idx_p.reshape(CORES, T, 128)
    off_p = off_p.reshape(CORES, T, 128)
    val_p = val_p.reshape(CORES, T, 128)

    # off/val tables: [core, 128, T]
    off_tab = np.ascontiguousarray(off_p.transpose(0, 2, 1))
    val_tab = np.ascontiguousarray(val_p.transpose(0, 2, 1))

    # wrapped int16 gather indices, call-major: [core, 128, T*8]
    seg = idx_p.reshape(CORES, T * 128 // 16, 16)
    wrapped16 = seg.transpose(0, 2, 1)  # [CORES, 16, T*8]
    gidx_w = np.ascontiguousarray(np.tile(wrapped16, (1, 8, 1)))  # [CORES, 128, T*8]

    return dict(Bw=Bw, cstart=cstart, calls=calls, T=T,
                off_tab=off_tab, val_tab=val_tab, gidx_w=gidx_w)


def _build_program(plan):
    import concourse.bass as bass
    import concourse.bacc as bacc
    import concourse.mybir as mybir
    import concourse.tile as tile

    dt = mybir.dt
    AluOp = mybir.AluOpType
    Bw, cstart, calls, T = plan["Bw"], plan["cstart"], plan["calls"], plan["T"]

    nc = bacc.Bacc("TRN2", target_bir_lowering=False, debug=False,
                   num_devices=CORES)

    # ---- I/O ----
    xT_d = nc.dram_tensor("xT", [128, RPC], dt.float32, kind="ExternalInput")
    wb_d = nc.dram_tensor("wb", [128, 384 + 384], dt.float32, kind="ExternalInput")
    # wb = [W_packed[128,384] | biasrow: row0 = b0|b1|b2 broadcast along partitions]
    iota_d = nc.dram_tensor("iota", [128, 128], dt.float32, kind="ExternalInput")
    off_d = nc.dram_tensor("offt", [128, T], dt.float32, kind="ExternalInput")
    val_d = nc.dram_tensor("valt", [128, T], dt.float32, kind="ExternalInput")
    gix_d = nc.dram_tensor("gixt", [128, T * 8], dt.int16, kind="ExternalInput")
    out_d = nc.dram_tensor("out", [RPC, 384], dt.float32, kind="ExternalOutput")

    with tile.TileContext(nc) as tc:
        with (
            tc.tile_pool(name="const", bufs=1) as constp,
            tc.tile_pool(name="gath", bufs=2) as gathp,
            tc.tile_pool(name="pt", bufs=6) as ptp,
            tc.tile_pool(name="ev", bufs=6) as evp,
            tc.tile_pool(name="psum", bufs=4, space="PSUM") as psp,
            tc.tile_pool(name="psd", bufs=2, space="PSUM") as psdp,
            tc.tile_pool(name="dram", bufs=1, space="DRAM") as dramp,
        ):
            # ---- constants to SBUF ----
            xT = constp.tile([128, RPC], dt.float32)
            nc.sync.dma_start(xT[:], xT_d[:])
            wb = constp.tile([128, 768], dt.float32)
            nc.sync.dma_start(wb[:], wb_d[:])
            iota = constp.tile([128, 128], dt.float32)
            nc.sync.dma_start(iota[:], iota_d[:])
            offt = constp.tile([128, T], dt.float32)
            nc.sync.dma_start(offt[:], off_d[:])
            valt = constp.tile([128, T], dt.float32)
            nc.sync.dma_start(valt[:], val_d[:])
            gixt = constp.tile([128, T * 8], dt.int16)
            nc.sync.dma_start(gixt[:], gix_d[:])
            ones = constp.tile([1, 128], dt.float32)
            nc.vector.memset(ones[:], 1.0)

            # ---- internal DRAM ----
            hcat_sh = dramp.tile([RPC // 2, 2, 2, 128], dt.bfloat16)
            hcat_fl = dramp.tile([NP // 2, 512], dt.bfloat16, addr_space="Shared")
            g_sh = dramp.tile([RPC // 2, 2, 128], dt.bfloat16)
            g_fl = dramp.tile([NP // 2, 256], dt.bfloat16, addr_space="Shared")

            # ---- dense phase: h0 -> out, h1/h2 -> hcat_sh ----
            for w in range(NW):
                ph = psdp.tile([128, 384], dt.float32, tag="ph")
                # bias seed: ones^T @ [b0|b1|b2]  (broadcast biases into PSUM)
                nc.tensor.matmul(ph[:], ones[:], wb[0:1, 384:768],
                                 start=True, stop=False)
                for j in range(3):
                    nc.tensor.matmul(ph[:, j * 128:(j + 1) * 128],
                                     xT[:, w * 128:(w + 1) * 128],
                                     wb[:, j * 128:(j + 1) * 128],
                                     start=False, stop=(j == 2))
                h0 = evp.tile([128, 128], dt.float32, tag="h0")
                nc.scalar.copy(h0[:], ph[:, 0:128])
                nc.sync.dma_start(out_d[w * 128:(w + 1) * 128, 0:128], h0[:])
                for j in (1, 2):
                    hj = evp.tile([128, 128], dt.bfloat16, tag="hj")
                    nc.scalar.copy(hj[:], ph[:, j * 128:(j + 1) * 128])
                    nc.sync.dma_start(
                        hcat_sh[w * 64:(w + 1) * 64, :, j - 1, :], hj[:])

            # ---- AllGather hcat ----
            if STAGES >= 2:
                nc.gpsimd.collective_compute(
                    "AllGather", mybir.AluOpType.bypass,
                    replica_groups=[list(range(CORES))],
                    ins=[hcat_sh[:].opt()], outs=[hcat_fl[:].opt()])

            # ---- SpMM passes ----
            def spmm_pass(src_fl, elem, gdt, out_cols, evict_g):
                for (w0, w1) in GROUPS:
                    gts = {}
                    for p in (0, 1):
                        call = calls[(w0 // SG) * 2 + p]
                        assert call["par"] == p and call["ws"][0] == w0
                        nch = call["nch"]
                        cs = call["cstart"]
                        gt = gathp.tile([128, nch, elem], gdt,
                                        tag=f"g{p}", bufs=2)
                        nc.gpsimd.dma_gather(
                            gt[:], src_fl[:, p * elem:(p + 1) * elem],
                            gixt[:, cs * 8:(cs + nch) * 8],
                            num_idxs=nch * 128, num_idxs_reg=nch * 128,
                            elem_size=elem, elem_step=2 * elem,
                            single_packet=False)
                        gts[p] = (gt, cs)
                    for w in range(w0, w1):
                        nchw = int(Bw[w, 0] + Bw[w, 1])
                        ps = psp.tile([128, out_cols], dt.float32, tag="ps")
                        k = 0
                        for p in (0, 1):
                            gt, cs = gts[p]
                            for b in range(int(Bw[w, p])):
                                cg = int(cstart[w, p]) + b
                                lp = cg - cs
                                pt = ptp.tile([128, 128], gdt, tag="pt")
                                nc.vector.tensor_scalar(
                                    pt[:], iota[:],
                                    offt[:, cg:cg + 1], valt[:, cg:cg + 1],
                                    AluOp.is_equal, AluOp.mult)
                                nc.tensor.matmul(
                                    ps[:], pt[:], gt[:, lp, :],
                                    start=(k == 0), stop=(k == nchw - 1))
                                k += 1
                        # evict
                        yc = evp.tile([128, 128], dt.float32, tag="yc")
                        nc.scalar.copy(yc[:], ps[:, 0:128])
                        nc.sync.dma_start(
                            out_d[w * 128:(w + 1) * 128,
                                  out_cols_off:out_cols_off + 128], yc[:])
                        if evict_g:
                            gc = evp.tile([128, 128], dt.bfloat16, tag="gc")
                            nc.scalar.copy(gc[:], ps[:, 128:256])
                            nc.sync.dma_start(g_sh[w * 64:(w + 1) * 64, :, :],
                                              gc[:])

            out_cols_off = 128
            if STAGES >= 3:
                spmm_pass(hcat_fl, 256, dt.bfloat16, 256, True)

            # ---- AllGather g ----
            if STAGES >= 4:
                nc.gpsimd.collective_compute(
                    "AllGather", mybir.AluOpType.bypass,
                    replica_groups=[list(range(CORES))],
                    ins=[g_sh[:].opt()], outs=[g_fl[:].opt()])

            out_cols_off = 256
            if STAGES >= 5:
                spmm_pass(g_fl, 128, dt.bfloat16, 128, False)

    nc.compile()
    return nc


def _prepare_inputs(x, W, b, plan):
    xpad = np.zeros((NP, C), np.float32)
    xpad[:N] = x
    xT = xpad.T  # [128, NP]
    Wp = np.concatenate([W[0], W[1], W[2]], axis=1)  # [128, 384]
    biasrow = np.zeros((128, 384), np.float32)
    biasrow[0] = np.concatenate([b[0], b[1], b[2]])
    wb = np.concatenate([Wp, biasrow], axis=1)  # [128, 768]
    iota = np.broadcast_to(np.arange(128, dtype=np.float32), (128, 128)).copy()

    in_maps = []
    for c in range(CORES):
        in_maps.append({
            "xT": np.ascontiguousarray(xT[:, c * RPC:(c + 1) * RPC]),
            "wb": wb,
            "iota": iota,
            "offt": plan["off_tab"][c],
            "valt": plan["val_tab"][c],
            "gixt": plan["gidx_w"][c],
        })
    return in_maps


def kernel(x, W, b, edge_val, edge_row, edge_col):
    x = np.asarray(x, np.float32)
    W = np.asarray(W, np.float32)
    b = np.asarray(b, np.float32)
    edge_val = np.asarray(edge_val, np.float32)
    edge_row = np.asarray(edge_row, np.int32)
    edge_col = np.asarray(edge_col, np.int32)

    from concourse.bass_utils import run_bass_kernel_spmd

    key = hash((edge_row.tobytes(), edge_col.tobytes()))
    if key not in _CACHE:
        plan = _build_plan(edge_row, edge_col, edge_val)
        nc = _build_program(plan)
        _CACHE[key] = (plan, nc)
    plan, nc = _CACHE[key]
    # val table depends on edge_val; rebuild if needed (cheap) — plan built
    # with the same edge_val in the common path.

    in_maps = _prepare_inputs(x, W, b, plan)
    res = run_bass_kernel_spmd(nc, in_maps, core_ids=list(range(CORES)),
                               trace=TRACE)
    kernel.last_results = res
    out = np.concatenate([res.results[c]["out"] for c in range(CORES)], axis=0)
    return np.ascontiguousarray(out[:N])


if __name__ == "__main__":
    rng = np.random.default_rng(0)
    x = rng.standard_normal((N, C), dtype=np.float32)
    W = rng.standard_normal((3, C, C), dtype=np.float32) / np.sqrt(C)
    b = rng.standard_normal((3, C), dtype=np.float32) * 0.01
    ev = rng.random(E, dtype=np.float32)
    er = rng.integers(0, N, E, dtype=np.int32)
    ec = rng.integers(0, N, E, dtype=np.int32)
    out = kernel(x=x, W=W, b=b, edge_val=ev, edge_row=er, edge_col=ec)
    print(out.shape, out.dtype)


# revision 6
# speedup vs baseline: 1.4193x; 1.4193x over previous
"""MixHop layer (hop0 + A@h1 + A^2@h2) on 8 trn2 NeuronCores.

Strategy: 1D node partition (rows) across 8 cores. Dense hop matmuls on
TensorE. SpMM = dma_gather of neighbor features (bf16, 512B rows, 4 SWDGE
queues) + one-hot scatter matmuls on TensorE accumulating into
per-128-row-window PSUM tiles. The one-hot-scaled stationary tile
P_T[e, r] = val_e * (r == row_off_e) is precomputed on the host (bf16) and
streamed in with large DMAs. Cross-core halo handled by two AllGathers
(hcat=[h1|h2] bf16, g bf16).
"""
import os
import sys

for p in ("/opt/trn_rl_repo", "/root/.axon_site/_ro/trn_rl_repo"):
    if os.path.isdir(p) and p not in sys.path:
        sys.path.append(p)

import numpy as np
import ml_dtypes

N = 50000
E = 600000
C = 128
CORES = 8
RPC = 6272                # rows per core (padded)
NP = RPC * CORES          # 50176
NW = RPC // 128           # 49 windows per core
SG = 6                    # windows per gather supergroup
GROUPS = [(g * SG, min(NW, (g + 1) * SG)) for g in range((NW + SG - 1) // SG)]
NQ = 4                    # SWDGE queues

TRACE = False             # test.py can flip this for profiling
STAGES = int(os.environ.get("KM_STAGES", "5"))
_CACHE = {}


def _build_plan(edge_row, edge_col, edge_val):
    """Host-side edge preprocessing. Returns per-core tables + structure."""
    core = edge_row // RPC
    w = (edge_row % RPC) // 128
    off = (edge_row % 128).astype(np.int64)
    par = (edge_col % 2).astype(np.int64)
    gidx = (edge_col // 2).astype(np.int16)

    gid = (core.astype(np.int64) * NW + w) * 2 + par
    ngroups = CORES * NW * 2
    counts = np.bincount(gid, minlength=ngroups).reshape(CORES, NW, 2)
    Bw = np.maximum(1, ((counts.max(axis=0) + 127) // 128))  # [NW, 2]

    # global chunk layout in "call order": for group: for par: for w in group
    cstart = np.zeros((NW, 2), np.int64)
    calls = []
    cpos = 0
    for (w0, w1) in GROUPS:
        for p in (0, 1):
            ws = list(range(w0, w1))
            nch = int(Bw[w0:w1, p].sum())
            for wi in ws:
                cstart[wi, p] = cpos
                cpos += int(Bw[wi, p])
            calls.append(dict(par=p, ws=ws, cstart=cpos - nch, nch=nch))
    T = cpos

    order = np.argsort(gid, kind="stable")
    gs = np.zeros(ngroups + 1, np.int64)
    np.cumsum(counts.reshape(-1), out=gs[1:])
    rank = np.arange(E, dtype=np.int64) - gs[gid[order]]
    pos = cstart[w[order], par[order]] * 128 + rank
    flat = core[order].astype(np.int64) * (T * 128) + pos

    idx_p = np.zeros(CORES * T * 128, np.int16)
    idx_p[flat] = gidx[order]
    idx_p = idx_p.reshape(CORES, T, 128)

    # one-hot P_T tiles: [core, T*128 edges, 128 rows] bf16, then e-major
    pt = np.zeros((CORES * T * 128, 128), ml_dtypes.bfloat16)
    pt[flat, off[order]] = edge_val[order].astype(ml_dtypes.bfloat16)
    pt = pt.reshape(CORES, T, 128, 128).transpose(0, 2, 1, 3)
    pt = np.ascontiguousarray(pt.reshape(CORES, 128, T * 128))

    # wrapped int16 gather indices, call-major: [core, 128, T*8]
    seg = idx_p.reshape(CORES, T * 128 // 16, 16)
    wrapped16 = seg.transpose(0, 2, 1)  # [CORES, 16, T*8]
    gidx_w = np.ascontiguousarray(np.tile(wrapped16, (1, 8, 1)))

    return dict(Bw=Bw, cstart=cstart, calls=calls, T=T,
                pt=pt, gidx_w=gidx_w)


def _build_program(plan):
    import concourse.bass as bass
    import concourse.bacc as bacc
    import concourse.mybir as mybir
    import concourse.tile as tile

    dt = mybir.dt
    Bw, cstart, calls, T = plan["Bw"], plan["cstart"], plan["calls"], plan["T"]

    nc = bacc.Bacc("TRN2", target_bir_lowering=False, debug=False,
                   num_devices=CORES, num_swdge_queues=NQ)

    xT_d = nc.dram_tensor("xT", [128, RPC], dt.float32, kind="ExternalInput")
    wb_d = nc.dram_tensor("wb", [128, 768], dt.float32, kind="ExternalInput")
    pt_d = nc.dram_tensor("ptt", [128, T * 128], dt.bfloat16, kind="ExternalInput")
    gix_d = nc.dram_tensor("gixt", [128, T * 8], dt.int16, kind="ExternalInput")
    out_d = nc.dram_tensor("out", [RPC, 384], dt.float32, kind="ExternalOutput")

    qn = [0]  # round-robin SWDGE queue counter

    with tile.TileContext(nc) as tc:
        with (
            tc.tile_pool(name="const", bufs=1) as constp,
            tc.tile_pool(name="gath", bufs=2) as gathp,
            tc.tile_pool(name="pt", bufs=2) as ptp,
            tc.tile_pool(name="ev", bufs=6) as evp,
            tc.tile_pool(name="psum", bufs=4, space="PSUM") as psp,
            tc.tile_pool(name="psd", bufs=2, space="PSUM") as psdp,
            tc.tile_pool(name="dram", bufs=1, space="DRAM") as dramp,
        ):
            xT = constp.tile([128, RPC], dt.float32)
            nc.sync.dma_start(xT[:], xT_d[:])
            wb = constp.tile([128, 768], dt.float32)
            nc.sync.dma_start(wb[:], wb_d[:])
            gixt = constp.tile([128, T * 8], dt.int16)
            nc.sync.dma_start(gixt[:], gix_d[:])
            ones = constp.tile([1, 128], dt.float32)
            nc.vector.memset(ones[:], 1.0)

            hcat_sh = dramp.tile([RPC // 2, 2, 2, 128], dt.bfloat16)
            hcat_fl = dramp.tile([NP // 2, 512], dt.bfloat16, addr_space="Shared")
            g_sh = dramp.tile([RPC // 2, 2, 128], dt.bfloat16)
            g_fl = dramp.tile([NP // 2, 256], dt.bfloat16, addr_space="Shared")

            # ---- dense phase: h0 -> out, h1/h2 -> hcat_sh ----
            for w in range(NW):
                ph = psdp.tile([128, 384], dt.float32, tag="ph")
                nc.tensor.matmul(ph[:], ones[:], wb[0:1, 384:768],
                                 start=True, stop=False)
                for j in range(3):
                    nc.tensor.matmul(ph[:, j * 128:(j + 1) * 128],
                                     xT[:, w * 128:(w + 1) * 128],
                                     wb[:, j * 128:(j + 1) * 128],
                                     start=False, stop=(j == 2))
                h0 = evp.tile([128, 128], dt.float32, tag="h0")
                nc.vector.tensor_copy(h0[:], ph[:, 0:128])
                nc.sync.dma_start(out_d[w * 128:(w + 1) * 128, 0:128], h0[:])
                for j in (1, 2):
                    hj = evp.tile([128, 128], dt.bfloat16, tag="hj")
                    nc.vector.tensor_copy(hj[:], ph[:, j * 128:(j + 1) * 128])
                    nc.sync.dma_start(
                        hcat_sh[w * 64:(w + 1) * 64, :, j - 1, :], hj[:])

            if STAGES >= 2:
                nc.gpsimd.collective_compute(
                    "AllGather", mybir.AluOpType.bypass,
                    replica_groups=[list(range(CORES))],
                    ins=[hcat_sh[:].opt()], outs=[hcat_fl[:].opt()])

            def spmm_pass(src_fl, elem, out_cols, out_off, evict_g):
                for (w0, w1) in GROUPS:
                    gi = w0 // SG
                    # one-hot stationary tiles for the whole supergroup
                    c0 = calls[gi * 2]["cstart"]
                    c1 = calls[gi * 2 + 1]["cstart"] + calls[gi * 2 + 1]["nch"]
                    ptt = ptp.tile([128, (c1 - c0) * 128], dt.bfloat16,
                                   tag="ptt")
                    nc.sync.dma_start(ptt[:], pt_d[:, c0 * 128:c1 * 128])
                    gts = {}
                    for p in (0, 1):
                        call = calls[gi * 2 + p]
                        nch = call["nch"]
                        cs = call["cstart"]
                        gt = gathp.tile([128, nch, elem], dt.bfloat16,
                                        tag=f"g{p}", bufs=2)
                        nc.gpsimd.dma_gather(
                            gt[:], src_fl[:, p * elem:(p + 1) * elem],
                            gixt[:, cs * 8:(cs + nch) * 8],
                            num_idxs=nch * 128, num_idxs_reg=nch * 128,
                            elem_size=elem, elem_step=2 * elem,
                            single_packet=False, queue_num=qn[0] % NQ)
                        qn[0] += 1
                        gts[p] = (gt, cs)
                    for w in range(w0, w1):
                        nchw = int(Bw[w, 0] + Bw[w, 1])
                        ps = psp.tile([128, out_cols], dt.float32, tag="ps")
                        k = 0
                        for p in (0, 1):
                            gt, cs = gts[p]
                            for bch in range(int(Bw[w, p])):
                                cg = int(cstart[w, p]) + bch
                                lp = cg - cs
                                nc.tensor.matmul(
                                    ps[:],
                                    ptt[:, (cg - c0) * 128:(cg - c0 + 1) * 128],
                                    gt[:, lp, :],
                                    start=(k == 0), stop=(k == nchw - 1))
                                k += 1
                        yc = evp.tile([128, 128], dt.float32, tag="yc")
                        nc.vector.tensor_copy(yc[:], ps[:, 0:128])
                        nc.sync.dma_start(
                            out_d[w * 128:(w + 1) * 128, out_off:out_off + 128],
                            yc[:])
                        if evict_g:
                            gc = evp.tile([128, 128], dt.bfloat16, tag="gc")
                            nc.vector.tensor_copy(gc[:], ps[:, 128:256])
                            nc.sync.dma_start(g_sh[w * 64:(w + 1) * 64, :, :],
                                              gc[:])

            if STAGES >= 3:
                spmm_pass(hcat_fl, 256, 256, 128, True)

            if STAGES >= 4:
                nc.gpsimd.collective_compute(
                    "AllGather", mybir.AluOpType.bypass,
                    replica_groups=[list(range(CORES))],
                    ins=[g_sh[:].opt()], outs=[g_fl[:].opt()])

            if STAGES >= 5:
                spmm_pass(g_fl, 128, 128, 256, False)

    nc.compile()
    return nc


def _prepare_inputs(x, W, b, plan):
    xpad = np.zeros((NP, C), np.float32)
    xpad[:N] = x
    xT = xpad.T
    Wp = np.concatenate([W[0], W[1], W[2]], axis=1)
    biasrow = np.zeros((128, 384), np.float32)
    biasrow[0] = np.concatenate([b[0], b[1], b[2]])
    wb = np.concatenate([Wp, biasrow], axis=1)

    in_maps = []
    for c in range(CORES):
        in_maps.append({
            "xT": np.ascontiguousarray(xT[:, c * RPC:(c + 1) * RPC]),
            "wb": wb,
            "ptt": plan["pt"][c],
            "gixt": plan["gidx_w"][c],
        })
    return in_maps


def kernel(x, W, b, edge_val, edge_row, edge_col):
    x = np.asarray(x, np.float32)
    W = np.asarray(W, np.float32)
    b = np.asarray(b, np.float32)
    edge_val = np.asarray(edge_val, np.float32)
    edge_row = np.asarray(edge_row, np.int32)
    edge_col = np.asarray(edge_col, np.int32)

    from concourse.bass_utils import run_bass_kernel_spmd

    key = hash((edge_row.tobytes(), edge_col.tobytes(), edge_val.tobytes()))
    if key not in _CACHE:
        plan = _build_plan(edge_row, edge_col, edge_val)
        nc = _build_program(plan)
        _CACHE[key] = (plan, nc)
    plan, nc = _CACHE[key]

    in_maps = _prepare_inputs(x, W, b, plan)
    res = run_bass_kernel_spmd(nc, in_maps, core_ids=list(range(CORES)),
                               trace=TRACE)
    kernel.last_results = res
    out = np.concatenate([res.results[c]["out"] for c in range(CORES)], axis=0)
    return np.ascontiguousarray(out[:N])


if __name__ == "__main__":
    rng = np.random.default_rng(0)
    x = rng.standard_normal((N, C), dtype=np.float32)
    W = rng.standard_normal((3, C, C), dtype=np.float32) / np.sqrt(C)
    b = rng.standard_normal((3, C), dtype=np.float32) * 0.01
    ev = rng.random(E, dtype=np.float32)
    er = rng.integers(0, N, E, dtype=np.int32)
    ec = rng.integers(0, N, E, dtype=np.int32)
    out = kernel(x=x, W=W, b=b, edge_val=ev, edge_row=er, edge_col=ec)
    print(out.shape, out.dtype)


# revision 10
# speedup vs baseline: 1.6585x; 1.1685x over previous
"""MixHop layer (hop0 + A@h1 + A^2@h2) on 8 trn2 NeuronCores.

Strategy: 1D node partition (rows) across 8 cores, with a host-side global
row permutation that load-balances edges across cores and 128-row windows
(output is inverse-permuted on the host). Dense hop matmuls on TensorE.
SpMM = dma_gather of neighbor features (bf16, 512B rows, 4 SWDGE queues) +
one-hot scatter matmuls on TensorE accumulating into per-window PSUM tiles.
The one-hot-scaled stationary tile P_T[e, r] = val_e * (r == row_off_e) is
precomputed on the host (bf16) and streamed in with large DMAs. Cross-core
halo handled by two AllGathers (hcat=[h1|h2] bf16, g bf16).
"""
import heapq
import os
import sys

for p in ("/opt/trn_rl_repo", "/root/.axon_site/_ro/trn_rl_repo"):
    if os.path.isdir(p) and p not in sys.path:
        sys.path.append(p)

import numpy as np
import ml_dtypes

N = 50000
E = 600000
C = 128
CORES = 8
NW = 50                   # windows per core
RPC = NW * 128            # 6400 rows per core (padded)
NP = RPC * CORES          # 51200
SG = 4                    # windows per gather supergroup
GROUPS = [(g * SG, min(NW, (g + 1) * SG)) for g in range((NW + SG - 1) // SG)]
NQ = 4                    # SWDGE queues
GBUFS = 3                 # gather tile buffers per parity

TRACE = False
STAGES = int(os.environ.get("KM_STAGES", "5"))
_CACHE = {}


def _balance_perm(edge_row, edge_col):
    """Assign nodes to (core, window) slots balancing per-(slot, parity)
    edge counts. Returns perm[new_pos] = old_row ... actually returns
    relabel[old_row] = new_row, where new_row = core*RPC + window*128 + k.
    """
    # per-node degree by destination (row) and parity of... we balance the
    # ROW side: window load = sum over rows of deg(row) split by col parity.
    # Parity of col after relabel is unknown until relabel is fixed -> use
    # total degree for balancing (parities stay ~50/50 per window).
    deg = np.bincount(edge_row, minlength=N).astype(np.int64)
    order = np.argsort(-deg, kind="stable")  # high degree first
    nslots = CORES * NW
    # greedy: put next node into least-loaded (core,window) with space
    loads = [(0, s) for s in range(nslots)]
    heapq.heapify(loads)
    space = np.full(nslots, 128, np.int64)
    new_of_old = np.empty(NP, np.int64)
    fill_ptr = np.zeros(nslots, np.int64)
    for r in order:
        while True:
            load, s = heapq.heappop(loads)
            if space[s] > 0:
                break
        k = 128 - space[s]
        space[s] -= 1
        new_of_old[r] = s * 128 + k
        if space[s] > 0:
            heapq.heappush(loads, (load + deg[r], s))
    # pad nodes fill remaining slots
    rem = []
    for s in range(nslots):
        for k in range(128 - space[s], 128):
            rem.append(s * 128 + k)
    new_of_old[N:] = rem
    return new_of_old


def _build_plan(edge_row, edge_col, edge_val):
    relabel = _balance_perm(edge_row, edge_col)
    er = relabel[edge_row]
    ec = relabel[edge_col]

    core = er // RPC
    w = (er % RPC) // 128
    off = (er % 128).astype(np.int64)
    par = (ec % 2).astype(np.int64)
    gidx = (ec // 2).astype(np.int16)

    gid = (core * NW + w) * 2 + par
    ngroups = CORES * NW * 2
    counts = np.bincount(gid, minlength=ngroups).reshape(CORES, NW, 2)
    Bw = np.maximum(1, ((counts.max(axis=0) + 127) // 128))  # [NW, 2]

    cstart = np.zeros((NW, 2), np.int64)
    calls = []
    cpos = 0
    for (w0, w1) in GROUPS:
        for p in (0, 1):
            ws = list(range(w0, w1))
            nch = int(Bw[w0:w1, p].sum())
            for wi in ws:
                cstart[wi, p] = cpos
                cpos += int(Bw[wi, p])
            calls.append(dict(par=p, ws=ws, cstart=cpos - nch, nch=nch))
    T = cpos

    order = np.argsort(gid, kind="stable")
    gs = np.zeros(ngroups + 1, np.int64)
    np.cumsum(counts.reshape(-1), out=gs[1:])
    rank = np.arange(E, dtype=np.int64) - gs[gid[order]]
    pos = cstart[w[order], par[order]] * 128 + rank
    flat = core[order] * (T * 128) + pos

    idx_p = np.zeros(CORES * T * 128, np.int16)
    idx_p[flat] = gidx[order]
    idx_p = idx_p.reshape(CORES, T, 128)

    pt = np.zeros((CORES * T * 128, 128), ml_dtypes.bfloat16)
    pt[flat, off[order]] = edge_val[order].astype(ml_dtypes.bfloat16)
    pt = pt.reshape(CORES, T, 128, 128).transpose(0, 2, 1, 3)
    pt = np.ascontiguousarray(pt.reshape(CORES, 128, T * 128))

    seg = idx_p.reshape(CORES, T * 128 // 16, 16)
    wrapped16 = seg.transpose(0, 2, 1)
    gidx_w = np.ascontiguousarray(np.tile(wrapped16, (1, 8, 1)))

    return dict(Bw=Bw, cstart=cstart, calls=calls, T=T,
                pt=pt, gidx_w=gidx_w, relabel=relabel)


def _build_program(plan):
    import concourse.bass as bass
    import concourse.bacc as bacc
    import concourse.mybir as mybir
    import concourse.tile as tile

    dt = mybir.dt
    Bw, cstart, calls, T = plan["Bw"], plan["cstart"], plan["calls"], plan["T"]

    nc = bacc.Bacc("TRN2", target_bir_lowering=False, debug=False,
                   num_devices=CORES, num_swdge_queues=NQ)

    xT_d = nc.dram_tensor("xT", [128, RPC], dt.float32, kind="ExternalInput")
    wb_d = nc.dram_tensor("wb", [128, 768], dt.float32, kind="ExternalInput")
    pt_d = nc.dram_tensor("ptt", [128, T * 128], dt.bfloat16, kind="ExternalInput")
    gix_d = nc.dram_tensor("gixt", [128, T * 8], dt.int16, kind="ExternalInput")
    out_d = nc.dram_tensor("out", [RPC, 384], dt.float32, kind="ExternalOutput")

    qn = [0]

    with tile.TileContext(nc) as tc:
        with (
            tc.tile_pool(name="const", bufs=1) as constp,
            tc.tile_pool(name="gath", bufs=GBUFS) as gathp,
            tc.tile_pool(name="pt", bufs=2) as ptp,
            tc.tile_pool(name="ev", bufs=2) as evp,
            tc.tile_pool(name="psum", bufs=4, space="PSUM") as psp,
            tc.tile_pool(name="psd", bufs=4, space="PSUM") as psdp,
            tc.tile_pool(name="dram", bufs=1, space="DRAM") as dramp,
        ):
            xT = constp.tile([128, RPC], dt.float32)
            nc.sync.dma_start(xT[:], xT_d[:])
            wb = constp.tile([128, 768], dt.float32)
            nc.sync.dma_start(wb[:], wb_d[:])
            gixt = constp.tile([128, T * 8], dt.int16)
            nc.sync.dma_start(gixt[:], gix_d[:])
            ones = constp.tile([1, 128], dt.float32)
            nc.vector.memset(ones[:], 1.0)

            hcat_sh = dramp.tile([RPC // 2, 2, 2, 128], dt.bfloat16)
            hcat_fl = dramp.tile([NP // 2, 512], dt.bfloat16, addr_space="Shared")
            g_sh = dramp.tile([RPC // 2, 2, 128], dt.bfloat16)
            g_fl = dramp.tile([NP // 2, 256], dt.bfloat16, addr_space="Shared")

            # ---- dense phase, batched per DG windows ----
            DG = 5
            for w0 in range(0, NW, DG):
                nwg = min(DG, NW - w0)
                h0b = evp.tile([128, nwg, 128], dt.float32, tag="h0")
                h1b = evp.tile([128, nwg, 128], dt.bfloat16, tag="h1")
                h2b = evp.tile([128, nwg, 128], dt.bfloat16, tag="h2")
                for wl in range(nwg):
                    w = w0 + wl
                    ph = psdp.tile([128, 384], dt.float32, tag="ph")
                    nc.tensor.matmul(ph[:], ones[:], wb[0:1, 384:768],
                                     start=True, stop=False)
                    for j in range(3):
                        nc.tensor.matmul(ph[:, j * 128:(j + 1) * 128],
                                         xT[:, w * 128:(w + 1) * 128],
                                         wb[:, j * 128:(j + 1) * 128],
                                         start=False, stop=(j == 2))
                    nc.vector.tensor_copy(h0b[:, wl, :], ph[:, 0:128])
                    nc.vector.tensor_copy(h1b[:, wl, :], ph[:, 128:256])
                    nc.vector.tensor_copy(h2b[:, wl, :], ph[:, 256:384])
                # h0 -> out rows [w0*128, (w0+nwg)*128), cols 0:128
                ov = out_d[w0 * 128:(w0 + nwg) * 128, 0:128].rearrange(
                    "(g p) c -> p g c", p=128)
                nc.sync.dma_start(ov, h0b[:])
                # h1/h2 -> hcat_sh[w0*64:(w0+nwg)*64, :, j-1, :]
                hv = hcat_sh[w0 * 64:(w0 + nwg) * 64, :, :, :].rearrange(
                    "(g a) b j c -> a b g j c", a=64)
                nc.sync.dma_start(hv[:, :, :, 0, :], h1b[:])
                nc.sync.dma_start(hv[:, :, :, 1, :], h2b[:])

            if STAGES >= 2:
                nc.gpsimd.collective_compute(
                    "AllGather", mybir.AluOpType.bypass,
                    replica_groups=[list(range(CORES))],
                    ins=[hcat_sh[:].opt()], outs=[hcat_fl[:].opt()])

            def spmm_pass(src_fl, elem, out_cols, out_off, evict_g):
                for (w0, w1) in GROUPS:
                    gi = w0 // SG
                    nwg = w1 - w0
                    c0 = calls[gi * 2]["cstart"]
                    c1 = calls[gi * 2 + 1]["cstart"] + calls[gi * 2 + 1]["nch"]
                    ptt = ptp.tile([128, (c1 - c0) * 128], dt.bfloat16,
                                   tag="ptt")
                    nc.sync.dma_start(ptt[:], pt_d[:, c0 * 128:c1 * 128])
                    gts = {}
                    for p in (0, 1):
                        call = calls[gi * 2 + p]
                        nch = call["nch"]
                        cs = call["cstart"]
                        gt = gathp.tile([128, nch, elem], dt.bfloat16,
                                        tag=f"g{p}")
                        nc.gpsimd.dma_gather(
                            gt[:], src_fl[:, p * elem:(p + 1) * elem],
                            gixt[:, cs * 8:(cs + nch) * 8],
                            num_idxs=nch * 128, num_idxs_reg=nch * 128,
                            elem_size=elem, elem_step=2 * elem,
                            single_packet=False, queue_num=qn[0] % NQ)
                        qn[0] += 1
                        gts[p] = (gt, cs)
                    ycb = evp.tile([128, nwg, 128], dt.float32, tag="yc")
                    if evict_g:
                        gcb = evp.tile([128, nwg, 128], dt.bfloat16, tag="gc")
                    for w in range(w0, w1):
                        nchw = int(Bw[w, 0] + Bw[w, 1])
                        ps = psp.tile([128, out_cols], dt.float32, tag="ps")
                        k = 0
                        for p in (0, 1):
                            gt, cs = gts[p]
                            for bch in range(int(Bw[w, p])):
                                cg = int(cstart[w, p]) + bch
                                lp = cg - cs
                                nc.tensor.matmul(
                                    ps[:],
                                    ptt[:, (cg - c0) * 128:(cg - c0 + 1) * 128],
                                    gt[:, lp, :],
                                    start=(k == 0), stop=(k == nchw - 1))
                                k += 1
                        nc.vector.tensor_copy(ycb[:, w - w0, :], ps[:, 0:128])
                        if evict_g:
                            nc.vector.tensor_copy(gcb[:, w - w0, :],
                                                  ps[:, 128:256])
                    ov = out_d[w0 * 128:w1 * 128,
                               out_off:out_off + 128].rearrange(
                        "(g p) c -> p g c", p=128)
                    nc.sync.dma_start(ov, ycb[:])
                    if evict_g:
                        gv = g_sh[w0 * 64:w1 * 64, :, :].rearrange(
                            "(g a) b c -> a b g c", a=64)
                        nc.sync.dma_start(gv[:], gcb[:])

            if STAGES >= 3:
                spmm_pass(hcat_fl, 256, 256, 128, True)

            if STAGES >= 4:
                nc.gpsimd.collective_compute(
                    "AllGather", mybir.AluOpType.bypass,
                    replica_groups=[list(range(CORES))],
                    ins=[g_sh[:].opt()], outs=[g_fl[:].opt()])

            if STAGES >= 5:
                spmm_pass(g_fl, 128, 128, 256, False)

    nc.compile()
    return nc


def _prepare_inputs(x, W, b, plan):
    relabel = plan["relabel"]
    xpad = np.zeros((NP, C), np.float32)
    xpad[relabel[:N]] = x
    xT = xpad.T
    Wp = np.concatenate([W[0], W[1], W[2]], axis=1)
    biasrow = np.zeros((128, 384), np.float32)
    biasrow[0] = np.concatenate([b[0], b[1], b[2]])
    wb = np.concatenate([Wp, biasrow], axis=1)

    in_maps = []
    for c in range(CORES):
        in_maps.append({
            "xT": np.ascontiguousarray(xT[:, c * RPC:(c + 1) * RPC]),
            "wb": wb,
            "ptt": plan["pt"][c],
            "gixt": plan["gidx_w"][c],
        })
    return in_maps


def kernel(x, W, b, edge_val, edge_row, edge_col):
    x = np.asarray(x, np.float32)
    W = np.asarray(W, np.float32)
    b = np.asarray(b, np.float32)
    edge_val = np.asarray(edge_val, np.float32)
    edge_row = np.asarray(edge_row, np.int32)
    edge_col = np.asarray(edge_col, np.int32)

    from concourse.bass_utils import run_bass_kernel_spmd

    key = hash((edge_row.tobytes(), edge_col.tobytes(), edge_val.tobytes()))
    if key not in _CACHE:
        plan = _build_plan(edge_row, edge_col, edge_val)
        nc = _build_program(plan)
        _CACHE[key] = (plan, nc)
    plan, nc = _CACHE[key]

    in_maps = _prepare_inputs(x, W, b, plan)
    res = run_bass_kernel_spmd(nc, in_maps, core_ids=list(range(CORES)),
                               trace=TRACE)
    kernel.last_results = res
    full = np.concatenate([res.results[c]["out"] for c in range(CORES)],
                          axis=0)
    return np.ascontiguousarray(full[plan["relabel"][:N]])


if __name__ == "__main__":
    rng = np.random.default_rng(0)
    x = rng.standard_normal((N, C), dtype=np.float32)
    W = rng.standard_normal((3, C, C), dtype=np.float32) / np.sqrt(C)
    b = rng.standard_normal((3, C), dtype=np.float32) * 0.01
    ev = rng.random(E, dtype=np.float32)
    er = rng.integers(0, N, E, dtype=np.int32)
    ec = rng.integers(0, N, E, dtype=np.int32)
    out = kernel(x=x, W=W, b=b, edge_val=ev, edge_row=er, edge_col=ec)
    print(out.shape, out.dtype)


# revision 11
# speedup vs baseline: 1.8751x; 1.1306x over previous
"""MixHop layer (hop0 + A@h1 + A^2@h2) on 8 trn2 NeuronCores.

Strategy: 1D node partition (rows) across 8 cores, with a host-side global
row permutation that load-balances edges across cores and 128-row windows
(output is inverse-permuted on the host). Dense hop matmuls on TensorE.
SpMM = dma_gather of neighbor features (bf16, 512B rows, 4 SWDGE queues) +
one-hot scatter matmuls on TensorE accumulating into per-window PSUM tiles.
The one-hot-scaled stationary tile P_T[e, r] = val_e * (r == row_off_e) is
precomputed on the host (bf16) and streamed in with large DMAs. Cross-core
halo handled by two AllGathers (hcat=[h1|h2] bf16, g bf16).
"""
import heapq
import os
import sys

for p in ("/opt/trn_rl_repo", "/root/.axon_site/_ro/trn_rl_repo"):
    if os.path.isdir(p) and p not in sys.path:
        sys.path.append(p)

import numpy as np
import ml_dtypes

N = 50000
E = 600000
C = 128
CORES = 8
NW = 50                   # windows per core
RPC = NW * 128            # 6400 rows per core (padded)
NP = RPC * CORES          # 51200
SG = 4                    # windows per gather supergroup
GROUPS = [(g * SG, min(NW, (g + 1) * SG)) for g in range((NW + SG - 1) // SG)]
NQ = 4                    # SWDGE queues
GBUFS = 4                 # gather tile buffers per parity

TRACE = False
STAGES = int(os.environ.get("KM_STAGES", "5"))
_CACHE = {}


def _balance_perm(edge_row, edge_col):
    """Assign nodes to (core, window) slots balancing per-(slot, parity)
    edge counts. Returns perm[new_pos] = old_row ... actually returns
    relabel[old_row] = new_row, where new_row = core*RPC + window*128 + k.
    """
    # per-node degree by destination (row) and parity of... we balance the
    # ROW side: window load = sum over rows of deg(row) split by col parity.
    # Parity of col after relabel is unknown until relabel is fixed -> use
    # total degree for balancing (parities stay ~50/50 per window).
    deg = np.bincount(edge_row, minlength=N).astype(np.int64)
    order = np.argsort(-deg, kind="stable")  # high degree first
    nslots = CORES * NW
    # greedy: put next node into least-loaded (core,window) with space
    loads = [(0, s) for s in range(nslots)]
    heapq.heapify(loads)
    space = np.full(nslots, 128, np.int64)
    new_of_old = np.empty(NP, np.int64)
    fill_ptr = np.zeros(nslots, np.int64)
    for r in order:
        while True:
            load, s = heapq.heappop(loads)
            if space[s] > 0:
                break
        k = 128 - space[s]
        space[s] -= 1
        new_of_old[r] = s * 128 + k
        if space[s] > 0:
            heapq.heappush(loads, (load + deg[r], s))
    # pad nodes fill remaining slots
    rem = []
    for s in range(nslots):
        for k in range(128 - space[s], 128):
            rem.append(s * 128 + k)
    new_of_old[N:] = rem
    return new_of_old


def _build_plan(edge_row, edge_col, edge_val):
    relabel = _balance_perm(edge_row, edge_col)
    er = relabel[edge_row]
    ec = relabel[edge_col]

    core = er // RPC
    w = (er % RPC) // 128
    off = (er % 128).astype(np.int64)
    par = (ec % 2).astype(np.int64)
    gidx = (ec // 2).astype(np.int16)

    gid = (core * NW + w) * 2 + par
    ngroups = CORES * NW * 2
    counts = np.bincount(gid, minlength=ngroups).reshape(CORES, NW, 2)
    Bw = np.maximum(1, ((counts.max(axis=0) + 127) // 128))  # [NW, 2]

    cstart = np.zeros((NW, 2), np.int64)
    calls = []
    cpos = 0
    for (w0, w1) in GROUPS:
        for p in (0, 1):
            ws = list(range(w0, w1))
            nch = int(Bw[w0:w1, p].sum())
            for wi in ws:
                cstart[wi, p] = cpos
                cpos += int(Bw[wi, p])
            calls.append(dict(par=p, ws=ws, cstart=cpos - nch, nch=nch))
    T = cpos

    order = np.argsort(gid, kind="stable")
    gs = np.zeros(ngroups + 1, np.int64)
    np.cumsum(counts.reshape(-1), out=gs[1:])
    rank = np.arange(E, dtype=np.int64) - gs[gid[order]]
    pos = cstart[w[order], par[order]] * 128 + rank
    flat = core[order] * (T * 128) + pos

    idx_p = np.zeros(CORES * T * 128, np.int16)
    idx_p[flat] = gidx[order]
    idx_p = idx_p.reshape(CORES, T, 128)

    pt = np.zeros((CORES * T * 128, 128), ml_dtypes.bfloat16)
    pt[flat, off[order]] = edge_val[order].astype(ml_dtypes.bfloat16)
    pt = pt.reshape(CORES, T, 128, 128).transpose(0, 2, 1, 3)
    pt = np.ascontiguousarray(pt.reshape(CORES, 128, T * 128))

    seg = idx_p.reshape(CORES, T * 128 // 16, 16)
    wrapped16 = seg.transpose(0, 2, 1)
    gidx_w = np.ascontiguousarray(np.tile(wrapped16, (1, 8, 1)))

    return dict(Bw=Bw, cstart=cstart, calls=calls, T=T,
                pt=pt, gidx_w=gidx_w, relabel=relabel)


def _build_program(plan):
    import concourse.bass as bass
    import concourse.bacc as bacc
    import concourse.mybir as mybir
    import concourse.tile as tile

    dt = mybir.dt
    Bw, cstart, calls, T = plan["Bw"], plan["cstart"], plan["calls"], plan["T"]

    nc = bacc.Bacc("TRN2", target_bir_lowering=False, debug=False,
                   num_devices=CORES, num_swdge_queues=NQ)

    xT_d = nc.dram_tensor("xT", [128, RPC], dt.bfloat16, kind="ExternalInput")
    wb_d = nc.dram_tensor("wb", [128, 768], dt.bfloat16, kind="ExternalInput")
    pt_d = nc.dram_tensor("ptt", [128, T * 128], dt.bfloat16, kind="ExternalInput")
    gix_d = nc.dram_tensor("gixt", [128, T * 8], dt.int16, kind="ExternalInput")
    out_d = nc.dram_tensor("out", [RPC, 384], dt.float32, kind="ExternalOutput")

    qn = [0]

    with tile.TileContext(nc) as tc:
        with (
            tc.tile_pool(name="const", bufs=1) as constp,
            tc.tile_pool(name="gath", bufs=GBUFS) as gathp,
            tc.tile_pool(name="pt", bufs=2) as ptp,
            tc.tile_pool(name="ev", bufs=2) as evp,
            tc.tile_pool(name="psum", bufs=4, space="PSUM") as psp,
            tc.tile_pool(name="psd", bufs=4, space="PSUM") as psdp,
            tc.tile_pool(name="dram", bufs=1, space="DRAM") as dramp,
        ):
            xT = constp.tile([128, RPC], dt.bfloat16)
            nc.sync.dma_start(xT[:], xT_d[:])
            wb = constp.tile([128, 768], dt.bfloat16)
            nc.sync.dma_start(wb[:], wb_d[:])
            gixt = constp.tile([128, T * 8], dt.int16)
            nc.sync.dma_start(gixt[:], gix_d[:])
            ones = constp.tile([1, 128], dt.bfloat16)
            nc.vector.memset(ones[:], 1.0)

            hcat_sh = dramp.tile([RPC // 2, 2, 2, 128], dt.bfloat16)
            hcat_fl = dramp.tile([NP // 2, 512], dt.bfloat16, addr_space="Shared")
            g_sh = dramp.tile([RPC // 2, 2, 128], dt.bfloat16)
            g_fl = dramp.tile([NP // 2, 256], dt.bfloat16, addr_space="Shared")

            # ---- dense phase, batched per DG windows ----
            DG = 5
            for w0 in range(0, NW, DG):
                nwg = min(DG, NW - w0)
                h0b = evp.tile([128, nwg, 128], dt.float32, tag="h0")
                h1b = evp.tile([128, nwg, 128], dt.bfloat16, tag="h1")
                h2b = evp.tile([128, nwg, 128], dt.bfloat16, tag="h2")
                for wl in range(nwg):
                    w = w0 + wl
                    ph = psdp.tile([128, 384], dt.float32, tag="ph")
                    nc.tensor.matmul(ph[:], ones[:], wb[0:1, 384:768],
                                     start=True, stop=False)
                    for j in range(3):
                        nc.tensor.matmul(ph[:, j * 128:(j + 1) * 128],
                                         xT[:, w * 128:(w + 1) * 128],
                                         wb[:, j * 128:(j + 1) * 128],
                                         start=False, stop=(j == 2))
                    nc.vector.tensor_copy(h0b[:, wl, :], ph[:, 0:128])
                    nc.vector.tensor_copy(h1b[:, wl, :], ph[:, 128:256])
                    nc.vector.tensor_copy(h2b[:, wl, :], ph[:, 256:384])
                # h0 -> out rows [w0*128, (w0+nwg)*128), cols 0:128
                ov = out_d[w0 * 128:(w0 + nwg) * 128, 0:128].rearrange(
                    "(g p) c -> p g c", p=128)
                nc.sync.dma_start(ov, h0b[:])
                # h1/h2 -> hcat_sh[w0*64:(w0+nwg)*64, :, j-1, :]
                hv = hcat_sh[w0 * 64:(w0 + nwg) * 64, :, :, :].rearrange(
                    "(g a) b j c -> a b g j c", a=64)
                nc.sync.dma_start(hv[:, :, :, 0, :], h1b[:])
                nc.sync.dma_start(hv[:, :, :, 1, :], h2b[:])

            if STAGES >= 2:
                nc.gpsimd.collective_compute(
                    "AllGather", mybir.AluOpType.bypass,
                    replica_groups=[list(range(CORES))],
                    ins=[hcat_sh[:].opt()], outs=[hcat_fl[:].opt()])

            def spmm_pass(src_fl, elem, out_cols, out_off, evict_g):
                for (w0, w1) in GROUPS:
                    gi = w0 // SG
                    nwg = w1 - w0
                    c0 = calls[gi * 2]["cstart"]
                    c1 = calls[gi * 2 + 1]["cstart"] + calls[gi * 2 + 1]["nch"]
                    ptt = ptp.tile([128, (c1 - c0) * 128], dt.bfloat16,
                                   tag="ptt")
                    nc.sync.dma_start(ptt[:], pt_d[:, c0 * 128:c1 * 128])
                    gts = {}
                    for p in (0, 1):
                        call = calls[gi * 2 + p]
                        nch = call["nch"]
                        cs = call["cstart"]
                        gt = gathp.tile([128, nch, elem], dt.bfloat16,
                                        tag=f"g{p}")
                        nc.gpsimd.dma_gather(
                            gt[:], src_fl[:, p * elem:(p + 1) * elem],
                            gixt[:, cs * 8:(cs + nch) * 8],
                            num_idxs=nch * 128, num_idxs_reg=nch * 128,
                            elem_size=elem, elem_step=2 * elem,
                            single_packet=False, queue_num=qn[0] % NQ)
                        qn[0] += 1
                        gts[p] = (gt, cs)
                    ycb = evp.tile([128, nwg, 128], dt.float32, tag="yc")
                    if evict_g:
                        gcb = evp.tile([128, nwg, 128], dt.bfloat16, tag="gc")
                    for w in range(w0, w1):
                        nchw = int(Bw[w, 0] + Bw[w, 1])
                        ps = psp.tile([128, out_cols], dt.float32, tag="ps")
                        k = 0
                        for p in (0, 1):
                            gt, cs = gts[p]
                            for bch in range(int(Bw[w, p])):
                                cg = int(cstart[w, p]) + bch
                                lp = cg - cs
                                nc.tensor.matmul(
                                    ps[:],
                                    ptt[:, (cg - c0) * 128:(cg - c0 + 1) * 128],
                                    gt[:, lp, :],
                                    start=(k == 0), stop=(k == nchw - 1))
                                k += 1
                        nc.vector.tensor_copy(ycb[:, w - w0, :], ps[:, 0:128])
                        if evict_g:
                            nc.vector.tensor_copy(gcb[:, w - w0, :],
                                                  ps[:, 128:256])
                    ov = out_d[w0 * 128:w1 * 128,
                               out_off:out_off + 128].rearrange(
                        "(g p) c -> p g c", p=128)
                    nc.sync.dma_start(ov, ycb[:])
                    if evict_g:
                        gv = g_sh[w0 * 64:w1 * 64, :, :].rearrange(
                            "(g a) b c -> a b g c", a=64)
                        nc.sync.dma_start(gv[:], gcb[:])

            if STAGES >= 3:
                spmm_pass(hcat_fl, 256, 256, 128, True)

            if STAGES >= 4:
                nc.gpsimd.collective_compute(
                    "AllGather", mybir.AluOpType.bypass,
                    replica_groups=[list(range(CORES))],
                    ins=[g_sh[:].opt()], outs=[g_fl[:].opt()])

            if STAGES >= 5:
                spmm_pass(g_fl, 128, 128, 256, False)

    nc.compile()
    return nc


def _prepare_inputs(x, W, b, plan):
    relabel = plan["relabel"]
    xpad = np.zeros((NP, C), np.float32)
    xpad[relabel[:N]] = x
    xT = xpad.T
    Wp = np.concatenate([W[0], W[1], W[2]], axis=1)
    biasrow = np.zeros((128, 384), np.float32)
    biasrow[0] = np.concatenate([b[0], b[1], b[2]])
    wb = np.concatenate([Wp, biasrow], axis=1)

    in_maps = []
    for c in range(CORES):
        in_maps.append({
            "xT": np.ascontiguousarray(xT[:, c * RPC:(c + 1) * RPC]).astype(ml_dtypes.bfloat16),
            "wb": wb.astype(ml_dtypes.bfloat16),
            "ptt": plan["pt"][c],
            "gixt": plan["gidx_w"][c],
        })
    return in_maps


def kernel(x, W, b, edge_val, edge_row, edge_col):
    x = np.asarray(x, np.float32)
    W = np.asarray(W, np.float32)
    b = np.asarray(b, np.float32)
    edge_val = np.asarray(edge_val, np.float32)
    edge_row = np.asarray(edge_row, np.int32)
    edge_col = np.asarray(edge_col, np.int32)

    from concourse.bass_utils import run_bass_kernel_spmd

    key = hash((edge_row.tobytes(), edge_col.tobytes(), edge_val.tobytes()))
    if key not in _CACHE:
        plan = _build_plan(edge_row, edge_col, edge_val)
        nc = _build_program(plan)
        _CACHE[key] = (plan, nc)
    plan, nc = _CACHE[key]

    in_maps = _prepare_inputs(x, W, b, plan)
    res = run_bass_kernel_spmd(nc, in_maps, core_ids=list(range(CORES)),
                               trace=TRACE)
    kernel.last_results = res
    full = np.concatenate([res.results[c]["out"] for c in range(CORES)],
                          axis=0)
    return np.ascontiguousarray(full[plan["relabel"][:N]])


if __name__ == "__main__":
    rng = np.random.default_rng(0)
    x = rng.standard_normal((N, C), dtype=np.float32)
    W = rng.standard_normal((3, C, C), dtype=np.float32) / np.sqrt(C)
    b = rng.standard_normal((3, C), dtype=np.float32) * 0.01
    ev = rng.random(E, dtype=np.float32)
    er = rng.integers(0, N, E, dtype=np.int32)
    ec = rng.integers(0, N, E, dtype=np.int32)
    out = kernel(x=x, W=W, b=b, edge_val=ev, edge_row=er, edge_col=ec)
    print(out.shape, out.dtype)
